# revision 17
# baseline (speedup 1.0000x reference)
"""Trainium2 Bass kernel for an 8-expert top-2 MoE layer (B=2,S=1024,D=768,FF=3072).

Strategy (expert-parallel over 8 NeuronCores):
  - Host: router matmul + softmax + top-2 + renormalized weights + the scalar
    dg gate (together ~0.1% of the FLOPs), then dispatch: gather each expert's
    assigned tokens into a fixed capacity-C buffer (one expert per core).
  - Device (per core): the expert FFN over its C tokens — h1/gate matmuls
    (SwiGLU), down-projection matmul, residual scale-add, LayerNorm, and the
    ln_g/ln_b affine.  Matmuls run in bf16 with fp32 PSUM accumulation.
  - Host: scatter-add the two expert contributions per token, compute
    expert_load from the routing.
"""
import os
import sys
sys.path.insert(0, "/opt/trn_rl_repo")

import ml_dtypes
import numpy as np

import concourse.bass as bass
import concourse.mybir as mybir
import concourse.tile as tile
from concourse import bacc
from concourse.bass_utils import run_bass_kernel_spmd

AF = mybir.ActivationFunctionType
ALU = mybir.AluOpType
F32 = mybir.dt.float32
BF16 = mybir.dt.bfloat16
NPBF16 = ml_dtypes.bfloat16

B, S, D = 2, 1024, 768
E = 8
FF = 4 * D
TOP_K = 2
LN_EPS = 1e-5
T = B * S
KC = D // 128     # 6 contraction chunks over D
NFT = FF // 128   # 24 partition tiles over FF
P = 128
W_BLK = 128       # f-columns of w1/w3 loaded per DMA block
NBLK = FF // W_BLK


def _free_chunks(c):
    """Split free dim c into matmul chunks of at most 512 (one PSUM bank)."""
    chunks = []
    rem = c
    while rem > 0:
        w = min(rem, 512)
        chunks.append(w)
        rem -= w
    return chunks


def _build_nc(C):
    """Per-core expert-FFN kernel with capacity C tokens (C multiple of 128)."""
    nct = C // P
    cch = _free_chunks(C)          # phase-B free-dim chunks over tokens
    dch = [384, 384]               # phase-C free-dim chunks over D

    nc = bacc.Bacc()
    xT = nc.declare_dram_parameter("xT", [P, KC, C], BF16, isOutput=False)
    xg = nc.declare_dram_parameter("xg", [P, C // P, D], F32, isOutput=False)
    sc = nc.declare_dram_parameter("sc", [C], F32, isOutput=False)
    w1 = nc.declare_dram_parameter("w1", [NBLK, P, KC, W_BLK], BF16, isOutput=False)
    w3 = nc.declare_dram_parameter("w3", [NBLK, P, KC, W_BLK], BF16, isOutput=False)
    w2 = nc.declare_dram_parameter("w2", [P, NFT, D], BF16, isOutput=False)
    w1b = nc.declare_dram_parameter("w1b", [FF], F32, isOutput=False)
    w3b = nc.declare_dram_parameter("w3b", [FF], F32, isOutput=False)
    lng = nc.declare_dram_parameter("lng", [D], F32, isOutput=False)
    lnb = nc.declare_dram_parameter("lnb", [D], F32, isOutput=False)
    yout = nc.declare_dram_parameter("yout", [C, D], F32, isOutput=True)

    def bcast(ap):
        return bass.AP(tensor=ap.tensor, offset=ap.offset, ap=[[0, P], *ap.ap])

    with tile.TileContext(nc) as tc:
        with tc.tile_pool(name="res", bufs=1) as res, \
             tc.tile_pool(name="wblk", bufs=4) as wblk, \
             tc.tile_pool(name="scr", bufs=2) as scr, \
             tc.tile_pool(name="psumB", bufs=2, space="PSUM") as pB, \
             tc.tile_pool(name="psumC", bufs=2, space="PSUM") as pC:

            # ---- loads needed by phase B (issued first) ----
            xT_sb = res.tile([P, KC, C], BF16)
            nc.scalar.dma_start(xT_sb[:, :KC // 2, :], xT[:, :KC // 2, :])
            nc.scalar.dma_start(xT_sb[:, KC // 2:, :], xT[:, KC // 2:, :])
            w1b_sb = res.tile([P, NFT], F32)
            nc.scalar.dma_start(w1b_sb[:], w1b.rearrange("(f p) -> p f", p=P))
            w3b_sb = res.tile([P, NFT], F32)
            nc.scalar.dma_start(w3b_sb[:], w3b.rearrange("(f p) -> p f", p=P))
            aT_sb = res.tile([P, NFT, C], BF16)
            w2_sb = res.tile([P, NFT, D], BF16)
            xg_sb = res.tile([P, nct, D], F32)
            sc_sb = res.tile([P, nct], F32)
            lng_sb = res.tile([P, D], F32)
            lnb_sb = res.tile([P, D], F32)

            # ---- phase B: aT[f, c] = silu(x@w1+b1) * silu(x@w3+b3), transposed
            n_blk = FF // W_BLK
            ft_per_blk = W_BLK // P
            for blk in range(n_blk):
                w1_t = wblk.tile([P, KC, W_BLK], BF16, tag="w1blk")
                w3_t = wblk.tile([P, KC, W_BLK], BF16, tag="w3blk")
                nc.sync.dma_start(w1_t[:], w1[blk, :, :, :])
                nc.sync.dma_start(w3_t[:], w3[blk, :, :, :])
                if blk == 8:
                    # gate the phase-C/D input loads behind phase-B progress so
                    # their DMA traffic stays out of the critical head window:
                    # the copies read aT (produced by this block), and the
                    # phase-C dma_starts below then WAR-wait on these writes
                    ft_gate = blk * ft_per_blk - 1
                    for dst in (w2_sb[:1, 0, :2], xg_sb[:1, 0, :2],
                                sc_sb[:1, :1], lng_sb[:1, :2], lnb_sb[:1, :2]):
                        nc.vector.tensor_copy(dst, aT_sb[:1, ft_gate, :dst.shape[-1]])
                for sub in range(ft_per_blk):
                    ft = blk * ft_per_blk + sub
                    th = scr.tile([P, C], F32, tag="th")
                    tg = scr.tile([P, C], F32, tag="tg")
                    for wt, bt, dst in ((w1_t, w1b_sb, th), (w3_t, w3b_sb, tg)):
                        off = 0
                        for ci, w in enumerate(cch):
                            ps = pB.tile([P, w], F32, tag=f"pb{ci}")
                            for kc in range(KC):
                                nc.tensor.matmul(
                                    ps[:],
                                    wt[:, kc, sub * P:(sub + 1) * P],
                                    xT_sb[:, kc, off:off + w],
                                    start=(kc == 0), stop=(kc == KC - 1))
                            nc.scalar.activation(dst[:, off:off + w], ps[:],
                                                 AF.Silu, bias=bt[:, ft:ft + 1])
                            off += w
                    nc.vector.tensor_tensor(aT_sb[:, ft, :], th[:], tg[:], ALU.mult)

            # ---- loads needed by phase C/D (gated behind phase-B blk 2) ----
            for ftg in range(0, NFT, 6):
                nc.sync.dma_start(w2_sb[:, ftg:ftg + 6, :], w2[:, ftg:ftg + 6, :])
            nc.sync.dma_start(xg_sb[:], xg[:, :, :])
            nc.sync.dma_start(sc_sb[:], sc.rearrange("(t p) -> p t", p=P))
            nc.gpsimd.dma_start(lng_sb[:], bcast(lng[:]))
            nc.gpsimd.dma_start(lnb_sb[:], bcast(lnb[:]))
            eps_sb = res.tile([P, 1], F32)
            nc.vector.memset(eps_sb[:], LN_EPS)

            # ---- phase C+D: ff = a @ w2; y = xg2 + sc*ff; LN; affine ----
            for ct in range(nct):
                pfs = []
                off = 0
                for i, w in enumerate(dch):
                    pf = pC.tile([P, w], F32, tag=("pfa", "pfb")[i])
                    for ft in range(NFT):
                        nc.tensor.matmul(pf[:],
                                         aT_sb[:, ft, ct * P:(ct + 1) * P],
                                         w2_sb[:, ft, off:off + w],
                                         start=(ft == 0), stop=(ft == NFT - 1))
                    pfs.append((pf, off, w))
                    off += w
                y = scr.tile([P, D], F32, tag="y")
                stats = scr.tile([P, 6, 6], F32, tag="stats")
                yv = y[:].rearrange("p (s g) -> p s g", g=P)
                for hi, (pf, off, w) in enumerate(pfs):
                    # y = ff * sc + xg2   (xg2 = xg + sc*w2_b, host-folded)
                    nc.vector.scalar_tensor_tensor(
                        y[:, off:off + w], pf[:], sc_sb[:, ct:ct + 1],
                        xg_sb[:, ct, off:off + w], ALU.mult, ALU.add)
                    for sg in range(off // P, (off + w) // P):
                        nc.vector.bn_stats(stats[:, sg, :], yv[:, sg, :])
                mv = scr.tile([P, 2], F32, tag="mv")
                nc.vector.bn_aggr(mv[:], stats[:])
                rstd = scr.tile([P, 1], F32, tag="rstd")
                nc.scalar.activation(rstd[:], mv[:, 1:2], AF.Sqrt,
                                     bias=eps_sb[:])
                nc.vector.reciprocal(rstd[:], rstd[:])
                u = scr.tile([P, D], F32, tag="u")
                nc.vector.tensor_scalar(u[:], y[:], mv[:, 0:1], rstd[:],
                                        ALU.subtract, ALU.mult)
                o = scr.tile([P, D], F32, tag="o")
                nc.vector.tensor_tensor(o[:], u[:], lng_sb[:], ALU.mult)
                nc.vector.tensor_tensor(o[:], o[:], lnb_sb[:], ALU.add)
                yr = yout.rearrange("(t p) d -> p t d", p=P)
                for q in range(0, D, 192):
                    nc.scalar.dma_start(yr[:, ct, q:q + 192], o[:, q:q + 192])

    nc.finalize()
    return nc


_NC_CACHE = {}
LAST_RESULTS = None


def _get_nc(C):
    if C not in _NC_CACHE:
        _NC_CACHE[C] = _build_nc(C)
    return _NC_CACHE[C]


def _ensure_ntff_hook():
    """Register the axon NTFF profile hook if this image's antenv lacks it,
    so run_bass_kernel_spmd(trace=True) works for profiling."""
    try:
        import antenv.axon_hooks  # noqa: F401
        return
    except ImportError:
        pass
    import types
    try:
        import antenv
    except ImportError:
        return
    mod = types.ModuleType("antenv.axon_hooks")
    _hook = [None]
    mod.set_axon_ntff_profile_hook = lambda h: _hook.__setitem__(0, h)
    mod.get_axon_ntff_profile_hook = lambda: _hook[0]
    sys.modules["antenv.axon_hooks"] = mod
    antenv.axon_hooks = mod
    try:
        from trn_agent_boot.trn_boot import _ntff_profile_via_ctypes
        h = _ntff_profile_via_ctypes("/opt/axon/libaxon_pjrt.so")
        if h is not None:
            mod.set_axon_ntff_profile_hook(h)
    except Exception:
        pass


def _arr_w13(w):
    """[D, FF] -> [NBLK, P, KC, W_BLK] bf16, contiguous per partition."""
    return np.ascontiguousarray(
        w.reshape(KC, P, NBLK, W_BLK).transpose(2, 1, 0, 3)).astype(NPBF16)


def _arr_w2(w):
    """[FF, D] -> [P, NFT, D] bf16."""
    return np.ascontiguousarray(
        w.reshape(NFT, P, D).transpose(1, 0, 2)).astype(NPBF16)


def _host_expert(xf_rows, e, w1_w, w1_b, w3_w, w3_b, w2_w, w2_b, dg_w, dg_b):
    """Exact (numpy f32) expert FFN for overflow tokens. Returns (ff, dg)."""
    x = xf_rows.astype(np.float32)
    h1 = x @ w1_w[e] + w1_b[e]
    g = x @ w3_w[e] + w3_b[e]
    silu = lambda v: v / (1.0 + np.exp(-v))
    ff = (silu(h1) * silu(g)) @ w2_w[e] + w2_b[e]
    dg = 1.0 / (1.0 + np.exp(-(x @ dg_w[e] + dg_b[e])))
    return ff, dg


def kernel(x, router_w, router_b, dg_w, dg_b, w1_w, w1_b, w2_w, w2_b,
           w3_w, w3_b, ln_g, ln_b):
    x = np.asarray(x, dtype=np.float32)
    router_w = np.asarray(router_w, dtype=np.float32)
    router_b = np.asarray(router_b, dtype=np.float32)
    dg_w = np.asarray(dg_w, dtype=np.float32)
    dg_b = np.asarray(dg_b, dtype=np.float32)
    w1_w = np.asarray(w1_w, dtype=np.float32)
    w1_b = np.asarray(w1_b, dtype=np.float32)
    w2_w = np.asarray(w2_w, dtype=np.float32)
    w2_b = np.asarray(w2_b, dtype=np.float32)
    w3_w = np.asarray(w3_w, dtype=np.float32)
    w3_b = np.asarray(w3_b, dtype=np.float32)
    ln_g = np.asarray(ln_g, dtype=np.float32)
    ln_b = np.asarray(ln_b, dtype=np.float32)

    xf = x.reshape(T, D)

    # ---- router (replicated math on host; ~0.04% of FLOPs) ----
    logits = xf @ router_w + router_b
    m = logits.max(axis=-1, keepdims=True)
    ex = np.exp(logits - m)
    routing = ex / ex.sum(axis=-1, keepdims=True)
    tok = np.arange(T)
    i1 = routing.argmax(axis=-1)
    r2 = routing.copy()
    r2[tok, i1] = -np.inf
    i2 = r2.argmax(axis=-1)
    v1 = routing[tok, i1]
    v2 = routing[tok, i2]
    e2 = np.exp(v2 - v1)
    tw1 = 1.0 / (1.0 + e2)
    tw2 = e2 / (1.0 + e2)

    # ---- dispatch ----
    idxs, wks = [], []
    for e in range(E):
        t1 = np.nonzero(i1 == e)[0]
        t2 = np.nonzero(i2 == e)[0]
        idxs.append(np.concatenate([t1, t2]))
        wks.append(np.concatenate([tw1[t1], tw2[t2]]).astype(np.float32))
    max_n = max(len(ix) for ix in idxs)
    C = min(768, max(512, -(-max_n // P) * P))
    nc = _get_nc(C)

    in_maps = []
    for e in range(E):
        n = min(len(idxs[e]), C)
        rows = idxs[e][:n]
        xg_e = np.zeros((C, D), dtype=np.float32)
        xg_e[:n] = xf[rows]
        # dg gate (scalar per token) on host; sc = dg * topk_weight
        dgv = 1.0 / (1.0 + np.exp(-(xg_e[:n] @ dg_w[e] + dg_b[e])))
        sc_e = np.zeros((C,), dtype=np.float32)
        sc_e[:n] = dgv * wks[e][:n]
        # fold w2 bias into the residual: y = (xg + sc*w2_b) + sc*ff
        xg2_e = xg_e + sc_e[:, None] * w2_b[e][None, :]
        xT_h = np.ascontiguousarray(
            xg_e.T.reshape(KC, P, C).transpose(1, 0, 2)).astype(NPBF16)
        xg2_h = np.ascontiguousarray(
            xg2_e.reshape(C // P, P, D).transpose(1, 0, 2))
        in_maps.append({
            "xT": xT_h,
            "xg": xg2_h,
            "sc": sc_e,
            "w1": _arr_w13(w1_w[e]),
            "w3": _arr_w13(w3_w[e]),
            "w2": _arr_w2(w2_w[e]),
            "w1b": np.ascontiguousarray(w1_b[e]),
            "w3b": np.ascontiguousarray(w3_b[e]),
            "lng": np.ascontiguousarray(ln_g[e]),
            "lnb": np.ascontiguousarray(ln_b[e]),
        })

    trace = os.environ.get("MOE_TRACE", "0") == "1"
    if trace:
        _ensure_ntff_hook()
    res = run_bass_kernel_spmd(nc, in_maps, core_ids=list(range(E)),
                               trace=trace,
                               tmpdir=os.environ.get("MOE_TRACE_DIR") or None)
    global LAST_RESULTS
    LAST_RESULTS = res

    out = np.zeros((T, D), dtype=np.float32)
    for e in range(E):
        n = min(len(idxs[e]), C)
        out[idxs[e][:n]] += res.results[e]["yout"][:n]
        if len(idxs[e]) > C:
            # capacity overflow (pathological routing): exact host math
            ov = idxs[e][C:]
            ffv, dgv = _host_expert(xf[ov], e, w1_w, w1_b, w3_w, w3_b,
                                    w2_w, w2_b, dg_w, dg_b)
            y = xf[ov] + (dgv * wks[e][C:])[:, None] * ffv
            mu = y.mean(axis=-1, keepdims=True)
            var = np.square(y - mu).mean(axis=-1, keepdims=True)
            out[ov] += ((y - mu) / np.sqrt(var + LN_EPS)) * ln_g[e] + ln_b[e]

    expert_load = np.bincount(np.concatenate([i1, i2]),
                              minlength=E).astype(np.float32)
    return out.reshape(B, S, D), expert_load


# revision 19
# speedup vs baseline: 1.1334x; 1.1334x over previous
"""Trainium2 Bass kernel for an 8-expert top-2 MoE layer (B=2,S=1024,D=768,FF=3072).

Strategy (expert-parallel over 8 NeuronCores):
  - Host: router matmul + softmax + top-2 + renormalized weights + the scalar
    dg gate (together ~0.1% of the FLOPs), then dispatch: gather each expert's
    assigned tokens into a fixed capacity-C buffer (one expert per core).
  - Device (per core): the expert FFN over its C tokens — h1/gate matmuls
    (SwiGLU), down-projection matmul, residual scale-add, LayerNorm, and the
    ln_g/ln_b affine.  Matmuls run in bf16 with fp32 PSUM accumulation.
  - Host: scatter-add the two expert contributions per token, compute
    expert_load from the routing.
"""
import os
import sys
sys.path.insert(0, "/opt/trn_rl_repo")

import ml_dtypes
import numpy as np

import concourse.bass as bass
import concourse.mybir as mybir
import concourse.tile as tile
from concourse import bacc
from concourse.bass_utils import run_bass_kernel_spmd

AF = mybir.ActivationFunctionType
ALU = mybir.AluOpType
F32 = mybir.dt.float32
BF16 = mybir.dt.bfloat16
NPBF16 = ml_dtypes.bfloat16

B, S, D = 2, 1024, 768
E = 8
FF = 4 * D
TOP_K = 2
LN_EPS = 1e-5
T = B * S
KC = D // 128     # 6 contraction chunks over D
NFT = FF // 128   # 24 partition tiles over FF
P = 128
W_BLK = 128       # f-columns of w1/w3 loaded per DMA block
NBLK = FF // W_BLK


def _free_chunks(c):
    """Split free dim c into matmul chunks of at most 512 (one PSUM bank)."""
    chunks = []
    rem = c
    while rem > 0:
        w = min(rem, 512)
        chunks.append(w)
        rem -= w
    return chunks


def _build_nc(C):
    """Per-core expert-FFN kernel with capacity C tokens (C multiple of 128)."""
    nct = -(-C // P)               # c-tiles (last one may be partial)
    cch = _free_chunks(C)          # phase-B free-dim chunks over tokens
    dch = [384, 384]               # phase-C free-dim chunks over D

    nc = bacc.Bacc()
    xT = nc.declare_dram_parameter("xT", [P, KC, C], BF16, isOutput=False)
    xg = nc.declare_dram_parameter("xg", [P, nct, D], F32, isOutput=False)
    sc = nc.declare_dram_parameter("sc", [nct * P], F32, isOutput=False)
    w1 = nc.declare_dram_parameter("w1", [NBLK, P, KC, W_BLK], BF16, isOutput=False)
    w3 = nc.declare_dram_parameter("w3", [NBLK, P, KC, W_BLK], BF16, isOutput=False)
    w2 = nc.declare_dram_parameter("w2", [P, NFT, D], BF16, isOutput=False)
    w1b = nc.declare_dram_parameter("w1b", [FF], F32, isOutput=False)
    w3b = nc.declare_dram_parameter("w3b", [FF], F32, isOutput=False)
    lng = nc.declare_dram_parameter("lng", [D], F32, isOutput=False)
    lnb = nc.declare_dram_parameter("lnb", [D], F32, isOutput=False)
    yout = nc.declare_dram_parameter("yout", [nct * P, D], F32, isOutput=True)

    def bcast(ap):
        return bass.AP(tensor=ap.tensor, offset=ap.offset, ap=[[0, P], *ap.ap])

    with tile.TileContext(nc) as tc:
        with tc.tile_pool(name="res", bufs=1) as res, \
             tc.tile_pool(name="wblk", bufs=6) as wblk, \
             tc.tile_pool(name="scr", bufs=2) as scr, \
             tc.tile_pool(name="psumB", bufs=2, space="PSUM") as pB, \
             tc.tile_pool(name="psumC", bufs=2, space="PSUM") as pC:

            # ---- loads needed by phase B (issued first) ----
            xT_sb = res.tile([P, KC, C], BF16)
            nc.sync.dma_start(xT_sb[:], xT[:, :, :])
            w1b_sb = res.tile([P, NFT], F32)
            nc.scalar.dma_start(w1b_sb[:], w1b.rearrange("(f p) -> p f", p=P))
            w3b_sb = res.tile([P, NFT], F32)
            nc.scalar.dma_start(w3b_sb[:], w3b.rearrange("(f p) -> p f", p=P))
            aT_sb = res.tile([P, NFT, C], BF16)
            w2_sb = res.tile([P, NFT, D], BF16)
            xg_sb = res.tile([P, nct, D], F32)
            sc_sb = res.tile([P, nct], F32)
            rows_of = lambda ct: min(P, C - ct * P)
            lng_sb = res.tile([P, D], F32)
            lnb_sb = res.tile([P, D], F32)

            # ---- phase B: aT[f, c] = silu(x@w1+b1) * silu(x@w3+b3), transposed
            n_blk = FF // W_BLK
            ft_per_blk = W_BLK // P
            for blk in range(n_blk):
                w1_t = wblk.tile([P, KC, W_BLK], BF16, tag="w1blk")
                w3_t = wblk.tile([P, KC, W_BLK], BF16, tag="w3blk")
                nc.sync.dma_start(w1_t[:], w1[blk, :, :, :])
                nc.sync.dma_start(w3_t[:], w3[blk, :, :, :])
                if blk == 8:
                    # gate the phase-C/D input loads behind phase-B progress so
                    # their DMA traffic stays out of the critical head window:
                    # the copies read aT (produced by this block), and the
                    # phase-C dma_starts below then WAR-wait on these writes
                    ft_gate = blk * ft_per_blk - 1
                    for dst in (w2_sb[:1, 0, :2], xg_sb[:1, 0, :2],
                                sc_sb[:1, :1], lng_sb[:1, :2], lnb_sb[:1, :2]):
                        nc.vector.tensor_copy(dst, aT_sb[:1, ft_gate, :dst.shape[-1]])
                for sub in range(ft_per_blk):
                    ft = blk * ft_per_blk + sub
                    th = scr.tile([P, C], F32, tag="th")
                    tg = scr.tile([P, C], F32, tag="tg")
                    for wt, bt, dst in ((w1_t, w1b_sb, th), (w3_t, w3b_sb, tg)):
                        off = 0
                        for ci, w in enumerate(cch):
                            ps = pB.tile([P, w], F32, tag=f"pb{ci}")
                            for kc in range(KC):
                                nc.tensor.matmul(
                                    ps[:],
                                    wt[:, kc, sub * P:(sub + 1) * P],
                                    xT_sb[:, kc, off:off + w],
                                    start=(kc == 0), stop=(kc == KC - 1))
                            nc.scalar.activation(dst[:, off:off + w], ps[:],
                                                 AF.Silu, bias=bt[:, ft:ft + 1])
                            off += w
                    nc.vector.tensor_tensor(aT_sb[:, ft, :], th[:], tg[:], ALU.mult)

            # ---- loads needed by phase C/D (gated behind phase-B blk 2) ----
            for ftg in range(0, NFT, 6):
                nc.sync.dma_start(w2_sb[:, ftg:ftg + 6, :], w2[:, ftg:ftg + 6, :])
            nc.sync.dma_start(xg_sb[:], xg[:, :, :])
            nc.sync.dma_start(sc_sb[:], sc.rearrange("(t p) -> p t", p=P))
            nc.gpsimd.dma_start(lng_sb[:], bcast(lng[:]))
            nc.gpsimd.dma_start(lnb_sb[:], bcast(lnb[:]))
            eps_sb = res.tile([P, 1], F32)
            nc.vector.memset(eps_sb[:], LN_EPS)

            # ---- phase C+D: ff = a @ w2; y = xg2 + sc*ff; LN; affine ----
            for ct in range(nct):
                rows = rows_of(ct)
                pfs = []
                off = 0
                for i, w in enumerate(dch):
                    pf = pC.tile([P, w], F32, tag=("pfa", "pfb")[i])
                    for ft in range(NFT):
                        nc.tensor.matmul(pf[:rows],
                                         aT_sb[:, ft, ct * P:ct * P + rows],
                                         w2_sb[:, ft, off:off + w],
                                         start=(ft == 0), stop=(ft == NFT - 1))
                    pfs.append((pf, off, w))
                    off += w
                y = scr.tile([P, D], F32, tag="y")
                stats = scr.tile([P, 6, 6], F32, tag="stats")
                yv = y[:].rearrange("p (s g) -> p s g", g=P)
                for hi, (pf, off, w) in enumerate(pfs):
                    # y = ff * sc + xg2   (xg2 = xg + sc*w2_b, host-folded)
                    nc.vector.scalar_tensor_tensor(
                        y[:rows, off:off + w], pf[:rows], sc_sb[:rows, ct:ct + 1],
                        xg_sb[:rows, ct, off:off + w], ALU.mult, ALU.add)
                    for sg in range(off // P, (off + w) // P):
                        nc.vector.bn_stats(stats[:rows, sg, :], yv[:rows, sg, :])
                mv = scr.tile([P, 2], F32, tag="mv")
                nc.vector.bn_aggr(mv[:rows], stats[:rows])
                rstd = scr.tile([P, 1], F32, tag="rstd")
                nc.scalar.activation(rstd[:rows], mv[:rows, 1:2], AF.Sqrt,
                                     bias=eps_sb[:rows])
                nc.vector.reciprocal(rstd[:rows], rstd[:rows])
                u = scr.tile([P, D], F32, tag="u")
                nc.vector.tensor_scalar(u[:rows], y[:rows], mv[:rows, 0:1],
                                        rstd[:rows], ALU.subtract, ALU.mult)
                o = scr.tile([P, D], F32, tag="o")
                nc.vector.tensor_tensor(o[:rows], u[:rows], lng_sb[:rows], ALU.mult)
                nc.vector.tensor_tensor(o[:rows], o[:rows], lnb_sb[:rows], ALU.add)
                yr = yout.rearrange("(t p) d -> p t d", p=P)
                for q in range(0, D, 192):
                    nc.sync.dma_start(yr[:rows, ct, q:q + 192],
                                      o[:rows, q:q + 192])

    nc.finalize()
    return nc


_NC_CACHE = {}
LAST_RESULTS = None


def _get_nc(C):
    if C not in _NC_CACHE:
        _NC_CACHE[C] = _build_nc(C)
    return _NC_CACHE[C]


def _ensure_ntff_hook():
    """Register the axon NTFF profile hook if this image's antenv lacks it,
    so run_bass_kernel_spmd(trace=True) works for profiling."""
    try:
        import antenv.axon_hooks  # noqa: F401
        return
    except ImportError:
        pass
    import types
    try:
        import antenv
    except ImportError:
        return
    mod = types.ModuleType("antenv.axon_hooks")
    _hook = [None]
    mod.set_axon_ntff_profile_hook = lambda h: _hook.__setitem__(0, h)
    mod.get_axon_ntff_profile_hook = lambda: _hook[0]
    sys.modules["antenv.axon_hooks"] = mod
    antenv.axon_hooks = mod
    try:
        from trn_agent_boot.trn_boot import _ntff_profile_via_ctypes
        h = _ntff_profile_via_ctypes("/opt/axon/libaxon_pjrt.so")
        if h is not None:
            mod.set_axon_ntff_profile_hook(h)
    except Exception:
        pass


def _arr_w13(w):
    """[D, FF] -> [NBLK, P, KC, W_BLK] bf16, contiguous per partition."""
    return np.ascontiguousarray(
        w.reshape(KC, P, NBLK, W_BLK).transpose(2, 1, 0, 3)).astype(NPBF16)


def _arr_w2(w):
    """[FF, D] -> [P, NFT, D] bf16."""
    return np.ascontiguousarray(
        w.reshape(NFT, P, D).transpose(1, 0, 2)).astype(NPBF16)


def _host_expert(xf_rows, e, w1_w, w1_b, w3_w, w3_b, w2_w, w2_b, dg_w, dg_b):
    """Exact (numpy f32) expert FFN for overflow tokens. Returns (ff, dg)."""
    x = xf_rows.astype(np.float32)
    h1 = x @ w1_w[e] + w1_b[e]
    g = x @ w3_w[e] + w3_b[e]
    silu = lambda v: v / (1.0 + np.exp(-v))
    ff = (silu(h1) * silu(g)) @ w2_w[e] + w2_b[e]
    dg = 1.0 / (1.0 + np.exp(-(x @ dg_w[e] + dg_b[e])))
    return ff, dg


def kernel(x, router_w, router_b, dg_w, dg_b, w1_w, w1_b, w2_w, w2_b,
           w3_w, w3_b, ln_g, ln_b):
    x = np.asarray(x, dtype=np.float32)
    router_w = np.asarray(router_w, dtype=np.float32)
    router_b = np.asarray(router_b, dtype=np.float32)
    dg_w = np.asarray(dg_w, dtype=np.float32)
    dg_b = np.asarray(dg_b, dtype=np.float32)
    w1_w = np.asarray(w1_w, dtype=np.float32)
    w1_b = np.asarray(w1_b, dtype=np.float32)
    w2_w = np.asarray(w2_w, dtype=np.float32)
    w2_b = np.asarray(w2_b, dtype=np.float32)
    w3_w = np.asarray(w3_w, dtype=np.float32)
    w3_b = np.asarray(w3_b, dtype=np.float32)
    ln_g = np.asarray(ln_g, dtype=np.float32)
    ln_b = np.asarray(ln_b, dtype=np.float32)

    xf = x.reshape(T, D)

    # ---- router (replicated math on host; ~0.04% of FLOPs) ----
    logits = xf @ router_w + router_b
    m = logits.max(axis=-1, keepdims=True)
    ex = np.exp(logits - m)
    routing = ex / ex.sum(axis=-1, keepdims=True)
    tok = np.arange(T)
    i1 = routing.argmax(axis=-1)
    r2 = routing.copy()
    r2[tok, i1] = -np.inf
    i2 = r2.argmax(axis=-1)
    v1 = routing[tok, i1]
    v2 = routing[tok, i2]
    e2 = np.exp(v2 - v1)
    tw1 = 1.0 / (1.0 + e2)
    tw2 = e2 / (1.0 + e2)

    # ---- dispatch ----
    idxs, wks = [], []
    for e in range(E):
        t1 = np.nonzero(i1 == e)[0]
        t2 = np.nonzero(i2 == e)[0]
        idxs.append(np.concatenate([t1, t2]))
        wks.append(np.concatenate([tw1[t1], tw2[t2]]).astype(np.float32))
    max_n = max(len(ix) for ix in idxs)
    C = min(768, max(512, -(-max_n // 64) * 64))
    nc = _get_nc(C)
    nct = -(-C // P)
    Cp = nct * P

    in_maps = []
    for e in range(E):
        n = min(len(idxs[e]), C)
        rows = idxs[e][:n]
        xg_e = np.zeros((Cp, D), dtype=np.float32)
        xg_e[:n] = xf[rows]
        # dg gate (scalar per token) on host; sc = dg * topk_weight
        dgv = 1.0 / (1.0 + np.exp(-(xg_e[:n] @ dg_w[e] + dg_b[e])))
        sc_e = np.zeros((Cp,), dtype=np.float32)
        sc_e[:n] = dgv * wks[e][:n]
        # fold w2 bias into the residual: y = (xg + sc*w2_b) + sc*ff
        xg2_e = xg_e + sc_e[:, None] * w2_b[e][None, :]
        xT_h = np.ascontiguousarray(
            xg_e[:C].T.reshape(KC, P, C).transpose(1, 0, 2)).astype(NPBF16)
        xg2_h = np.ascontiguousarray(
            xg2_e.reshape(nct, P, D).transpose(1, 0, 2))
        in_maps.append({
            "xT": xT_h,
            "xg": xg2_h,
            "sc": sc_e,
            "w1": _arr_w13(w1_w[e]),
            "w3": _arr_w13(w3_w[e]),
            "w2": _arr_w2(w2_w[e]),
            "w1b": np.ascontiguousarray(w1_b[e]),
            "w3b": np.ascontiguousarray(w3_b[e]),
            "lng": np.ascontiguousarray(ln_g[e]),
            "lnb": np.ascontiguousarray(ln_b[e]),
        })

    trace = os.environ.get("MOE_TRACE", "0") == "1"
    if trace:
        _ensure_ntff_hook()
    res = run_bass_kernel_spmd(nc, in_maps, core_ids=list(range(E)),
                               trace=trace,
                               tmpdir=os.environ.get("MOE_TRACE_DIR") or None)
    global LAST_RESULTS
    LAST_RESULTS = res

    out = np.zeros((T, D), dtype=np.float32)
    for e in range(E):
        n = min(len(idxs[e]), C)
        out[idxs[e][:n]] += res.results[e]["yout"][:n]
        if len(idxs[e]) > C:
            # capacity overflow (pathological routing): exact host math
            ov = idxs[e][C:]
            ffv, dgv = _host_expert(xf[ov], e, w1_w, w1_b, w3_w, w3_b,
                                    w2_w, w2_b, dg_w, dg_b)
            y = xf[ov] + (dgv * wks[e][C:])[:, None] * ffv
            mu = y.mean(axis=-1, keepdims=True)
            var = np.square(y - mu).mean(axis=-1, keepdims=True)
            out[ov] += ((y - mu) / np.sqrt(var + LN_EPS)) * ln_g[e] + ln_b[e]

    expert_load = np.bincount(np.concatenate([i1, i2]),
                              minlength=E).astype(np.float32)
    return out.reshape(B, S, D), expert_load


# revision 20
# speedup vs baseline: 1.3832x; 1.2204x over previous
"""Trainium2 Bass kernel for an 8-expert top-2 MoE layer (B=2,S=1024,D=768,FF=3072).

Strategy (expert-parallel over 8 NeuronCores):
  - Host: router matmul + softmax + top-2 + renormalized weights + the scalar
    dg gate (together ~0.1% of the FLOPs), then dispatch: gather each expert's
    assigned tokens into a fixed capacity-C buffer (one expert per core).
  - Device (per core): the expert FFN over its C tokens — h1/gate matmuls
    (SwiGLU), down-projection matmul, residual scale-add, LayerNorm, and the
    ln_g/ln_b affine.  Matmuls run in bf16 with fp32 PSUM accumulation.
  - Host: scatter-add the two expert contributions per token, compute
    expert_load from the routing.
"""
import os
import sys
sys.path.insert(0, "/opt/trn_rl_repo")

import ml_dtypes
import numpy as np

import concourse.bass as bass
import concourse.mybir as mybir
import concourse.tile as tile
from concourse import bacc
from concourse.bass_utils import run_bass_kernel_spmd

AF = mybir.ActivationFunctionType
ALU = mybir.AluOpType
F32 = mybir.dt.float32
BF16 = mybir.dt.bfloat16
NPBF16 = ml_dtypes.bfloat16
FP8 = mybir.dt.float8e4
NPFP8 = ml_dtypes.float8_e4m3
USE_FP8 = os.environ.get("MOE_FP8", "1") == "1"

B, S, D = 2, 1024, 768
E = 8
FF = 4 * D
TOP_K = 2
LN_EPS = 1e-5
T = B * S
KC = D // 128     # 6 contraction chunks over D
NFT = FF // 128   # 24 partition tiles over FF
P = 128
W_BLK = 128       # f-columns of w1/w3 loaded per DMA block
NBLK = FF // W_BLK


def _free_chunks(c):
    """Split free dim c into matmul chunks of at most 512 (one PSUM bank)."""
    chunks = []
    rem = c
    while rem > 0:
        w = min(rem, 512)
        chunks.append(w)
        rem -= w
    return chunks


def _build_nc(C, fp8=False):
    """Per-core expert-FFN kernel with capacity C tokens."""
    nct = -(-C // P)               # c-tiles (last one may be partial)
    cch = _free_chunks(C)          # phase-B free-dim chunks over tokens
    dch = [384, 384]               # phase-C free-dim chunks over D
    BDT = FP8 if fp8 else BF16     # dtype of the phase-B matmul operands

    nc = bacc.Bacc()
    xT = nc.declare_dram_parameter("xT", [P, KC, C], BDT, isOutput=False)
    xg = nc.declare_dram_parameter("xg", [P, nct, D], F32, isOutput=False)
    sc = nc.declare_dram_parameter("sc", [nct * P], F32, isOutput=False)
    w1 = nc.declare_dram_parameter("w1", [NBLK, P, KC, W_BLK], BDT, isOutput=False)
    w3 = nc.declare_dram_parameter("w3", [NBLK, P, KC, W_BLK], BDT, isOutput=False)
    desc = nc.declare_dram_parameter("desc", [2], F32, isOutput=False)
    w2 = nc.declare_dram_parameter("w2", [P, NFT, D], BF16, isOutput=False)
    w1b = nc.declare_dram_parameter("w1b", [FF], F32, isOutput=False)
    w3b = nc.declare_dram_parameter("w3b", [FF], F32, isOutput=False)
    lng = nc.declare_dram_parameter("lng", [D], F32, isOutput=False)
    lnb = nc.declare_dram_parameter("lnb", [D], F32, isOutput=False)
    yout = nc.declare_dram_parameter("yout", [nct * P, D], F32, isOutput=True)

    def bcast(ap):
        return bass.AP(tensor=ap.tensor, offset=ap.offset, ap=[[0, P], *ap.ap])

    with tile.TileContext(nc) as tc:
        with tc.tile_pool(name="res", bufs=1) as res, \
             tc.tile_pool(name="wblk", bufs=6) as wblk, \
             tc.tile_pool(name="scr", bufs=2) as scr, \
             tc.tile_pool(name="psumB", bufs=2, space="PSUM") as pB, \
             tc.tile_pool(name="psumC", bufs=2, space="PSUM") as pC:

            # ---- loads needed by phase B (issued first) ----
            xT_sb = res.tile([P, KC, C], BDT)
            nc.sync.dma_start(xT_sb[:], xT[:, :, :])
            desc_sb = res.tile([P, 2], F32)
            nc.gpsimd.dma_start(desc_sb[:], bcast(desc[:]))
            w1b_sb = res.tile([P, NFT], F32)
            nc.scalar.dma_start(w1b_sb[:], w1b.rearrange("(f p) -> p f", p=P))
            w3b_sb = res.tile([P, NFT], F32)
            nc.scalar.dma_start(w3b_sb[:], w3b.rearrange("(f p) -> p f", p=P))
            aT_sb = res.tile([P, NFT, C], BF16)
            w2_sb = res.tile([P, NFT, D], BF16)
            xg_sb = res.tile([P, nct, D], F32)
            sc_sb = res.tile([P, nct], F32)
            rows_of = lambda ct: min(P, C - ct * P)
            lng_sb = res.tile([P, D], F32)
            lnb_sb = res.tile([P, D], F32)

            # ---- phase B: aT[f, c] = silu(x@w1+b1) * silu(x@w3+b3), transposed
            n_blk = FF // W_BLK
            ft_per_blk = W_BLK // P
            for blk in range(n_blk):
                w1_t = wblk.tile([P, KC, W_BLK], BDT, tag="w1blk")
                w3_t = wblk.tile([P, KC, W_BLK], BDT, tag="w3blk")
                nc.sync.dma_start(w1_t[:], w1[blk, :, :, :])
                nc.sync.dma_start(w3_t[:], w3[blk, :, :, :])
                if blk == 8:
                    # gate the phase-C/D input loads behind phase-B progress so
                    # their DMA traffic stays out of the critical head window:
                    # the copies read aT (produced by this block), and the
                    # phase-C dma_starts below then WAR-wait on these writes
                    ft_gate = blk * ft_per_blk - 1
                    for dst in (w2_sb[:1, 0, :2], xg_sb[:1, 0, :2],
                                sc_sb[:1, :1], lng_sb[:1, :2], lnb_sb[:1, :2]):
                        nc.vector.tensor_copy(dst, aT_sb[:1, ft_gate, :dst.shape[-1]])
                for sub in range(ft_per_blk):
                    ft = blk * ft_per_blk + sub
                    th = scr.tile([P, C], F32, tag="th")
                    tg = scr.tile([P, C], F32, tag="tg")
                    for mi, (wt, bt, dst) in enumerate(
                            ((w1_t, w1b_sb, th), (w3_t, w3b_sb, tg))):
                        off = 0
                        for ci, w in enumerate(cch):
                            ps = pB.tile([P, w], F32, tag=f"pb{ci}")
                            if fp8 and w >= P:
                                # fp8 DoubleRow: contract two 128-k rows/pass
                                for kc2 in range(0, KC, 2):
                                    nc.tensor.matmul(
                                        ps[:],
                                        wt[:, kc2:kc2 + 2, sub * P:(sub + 1) * P],
                                        xT_sb[:, kc2:kc2 + 2, off:off + w],
                                        start=(kc2 == 0), stop=(kc2 == KC - 2),
                                        perf_mode=mybir.MatmulPerfMode.DoubleRow)
                            else:
                                for kc in range(KC):
                                    nc.tensor.matmul(
                                        ps[:],
                                        wt[:, kc, sub * P:(sub + 1) * P],
                                        xT_sb[:, kc, off:off + w],
                                        start=(kc == 0), stop=(kc == KC - 1))
                            scale = desc_sb[:, mi:mi + 1] if fp8 else 1.0
                            nc.scalar.activation(dst[:, off:off + w], ps[:],
                                                 AF.Silu, bias=bt[:, ft:ft + 1],
                                                 scale=scale)
                            off += w
                    nc.vector.tensor_tensor(aT_sb[:, ft, :], th[:], tg[:], ALU.mult)

            # ---- loads needed by phase C/D (gated behind phase-B blk 2) ----
            for ftg in range(0, NFT, 6):
                nc.sync.dma_start(w2_sb[:, ftg:ftg + 6, :], w2[:, ftg:ftg + 6, :])
            nc.sync.dma_start(xg_sb[:], xg[:, :, :])
            nc.sync.dma_start(sc_sb[:], sc.rearrange("(t p) -> p t", p=P))
            nc.gpsimd.dma_start(lng_sb[:], bcast(lng[:]))
            nc.gpsimd.dma_start(lnb_sb[:], bcast(lnb[:]))
            eps_sb = res.tile([P, 1], F32)
            nc.vector.memset(eps_sb[:], LN_EPS)

            # ---- phase C+D: ff = a @ w2; y = xg2 + sc*ff; LN; affine ----
            for ct in range(nct):
                rows = rows_of(ct)
                pfs = []
                off = 0
                for i, w in enumerate(dch):
                    pf = pC.tile([P, w], F32, tag=("pfa", "pfb")[i])
                    for ft in range(NFT):
                        nc.tensor.matmul(pf[:rows],
                                         aT_sb[:, ft, ct * P:ct * P + rows],
                                         w2_sb[:, ft, off:off + w],
                                         start=(ft == 0), stop=(ft == NFT - 1))
                    pfs.append((pf, off, w))
                    off += w
                y = scr.tile([P, D], F32, tag="y")
                stats = scr.tile([P, 6, 6], F32, tag="stats")
                yv = y[:].rearrange("p (s g) -> p s g", g=P)
                for hi, (pf, off, w) in enumerate(pfs):
                    # y = ff * sc + xg2   (xg2 = xg + sc*w2_b, host-folded)
                    nc.vector.scalar_tensor_tensor(
                        y[:rows, off:off + w], pf[:rows], sc_sb[:rows, ct:ct + 1],
                        xg_sb[:rows, ct, off:off + w], ALU.mult, ALU.add)
                    for sg in range(off // P, (off + w) // P):
                        nc.vector.bn_stats(stats[:rows, sg, :], yv[:rows, sg, :])
                mv = scr.tile([P, 2], F32, tag="mv")
                nc.vector.bn_aggr(mv[:rows], stats[:rows])
                rstd = scr.tile([P, 1], F32, tag="rstd")
                nc.scalar.activation(rstd[:rows], mv[:rows, 1:2], AF.Sqrt,
                                     bias=eps_sb[:rows])
                nc.vector.reciprocal(rstd[:rows], rstd[:rows])
                u = scr.tile([P, D], F32, tag="u")
                nc.vector.tensor_scalar(u[:rows], y[:rows], mv[:rows, 0:1],
                                        rstd[:rows], ALU.subtract, ALU.mult)
                o = scr.tile([P, D], F32, tag="o")
                nc.vector.tensor_tensor(o[:rows], u[:rows], lng_sb[:rows], ALU.mult)
                nc.vector.tensor_tensor(o[:rows], o[:rows], lnb_sb[:rows], ALU.add)
                yr = yout.rearrange("(t p) d -> p t d", p=P)
                for q in range(0, D, 192):
                    nc.sync.dma_start(yr[:rows, ct, q:q + 192],
                                      o[:rows, q:q + 192])

    nc.finalize()
    return nc


_NC_CACHE = {}
LAST_RESULTS = None


def _get_nc(C, fp8=False):
    if (C, fp8) not in _NC_CACHE:
        _NC_CACHE[(C, fp8)] = _build_nc(C, fp8)
    return _NC_CACHE[(C, fp8)]


def _pow2_scale(absmax):
    """Largest power of two s with absmax*s <= 224 (fp8e4m3 headroom)."""
    import math
    if absmax <= 0:
        return 1.0
    return 2.0 ** math.floor(math.log2(224.0 / absmax))


def _ensure_ntff_hook():
    """Register the axon NTFF profile hook if this image's antenv lacks it,
    so run_bass_kernel_spmd(trace=True) works for profiling."""
    try:
        import antenv.axon_hooks  # noqa: F401
        return
    except ImportError:
        pass
    import types
    try:
        import antenv
    except ImportError:
        return
    mod = types.ModuleType("antenv.axon_hooks")
    _hook = [None]
    mod.set_axon_ntff_profile_hook = lambda h: _hook.__setitem__(0, h)
    mod.get_axon_ntff_profile_hook = lambda: _hook[0]
    sys.modules["antenv.axon_hooks"] = mod
    antenv.axon_hooks = mod
    try:
        from trn_agent_boot.trn_boot import _ntff_profile_via_ctypes
        h = _ntff_profile_via_ctypes("/opt/axon/libaxon_pjrt.so")
        if h is not None:
            mod.set_axon_ntff_profile_hook(h)
    except Exception:
        pass


def _arr_w13(w):
    """[D, FF] -> [NBLK, P, KC, W_BLK], contiguous per partition."""
    dt = NPFP8 if USE_FP8 else NPBF16
    return np.ascontiguousarray(
        w.reshape(KC, P, NBLK, W_BLK).transpose(2, 1, 0, 3)).astype(dt)


def _arr_w2(w):
    """[FF, D] -> [P, NFT, D] bf16."""
    return np.ascontiguousarray(
        w.reshape(NFT, P, D).transpose(1, 0, 2)).astype(NPBF16)


def _host_expert(xf_rows, e, w1_w, w1_b, w3_w, w3_b, w2_w, w2_b, dg_w, dg_b):
    """Exact (numpy f32) expert FFN for overflow tokens. Returns (ff, dg)."""
    x = xf_rows.astype(np.float32)
    h1 = x @ w1_w[e] + w1_b[e]
    g = x @ w3_w[e] + w3_b[e]
    silu = lambda v: v / (1.0 + np.exp(-v))
    ff = (silu(h1) * silu(g)) @ w2_w[e] + w2_b[e]
    dg = 1.0 / (1.0 + np.exp(-(x @ dg_w[e] + dg_b[e])))
    return ff, dg


def kernel(x, router_w, router_b, dg_w, dg_b, w1_w, w1_b, w2_w, w2_b,
           w3_w, w3_b, ln_g, ln_b):
    x = np.asarray(x, dtype=np.float32)
    router_w = np.asarray(router_w, dtype=np.float32)
    router_b = np.asarray(router_b, dtype=np.float32)
    dg_w = np.asarray(dg_w, dtype=np.float32)
    dg_b = np.asarray(dg_b, dtype=np.float32)
    w1_w = np.asarray(w1_w, dtype=np.float32)
    w1_b = np.asarray(w1_b, dtype=np.float32)
    w2_w = np.asarray(w2_w, dtype=np.float32)
    w2_b = np.asarray(w2_b, dtype=np.float32)
    w3_w = np.asarray(w3_w, dtype=np.float32)
    w3_b = np.asarray(w3_b, dtype=np.float32)
    ln_g = np.asarray(ln_g, dtype=np.float32)
    ln_b = np.asarray(ln_b, dtype=np.float32)

    xf = x.reshape(T, D)

    # ---- router (replicated math on host; ~0.04% of FLOPs) ----
    logits = xf @ router_w + router_b
    m = logits.max(axis=-1, keepdims=True)
    ex = np.exp(logits - m)
    routing = ex / ex.sum(axis=-1, keepdims=True)
    tok = np.arange(T)
    i1 = routing.argmax(axis=-1)
    r2 = routing.copy()
    r2[tok, i1] = -np.inf
    i2 = r2.argmax(axis=-1)
    v1 = routing[tok, i1]
    v2 = routing[tok, i2]
    e2 = np.exp(v2 - v1)
    tw1 = 1.0 / (1.0 + e2)
    tw2 = e2 / (1.0 + e2)

    # ---- dispatch ----
    idxs, wks = [], []
    for e in range(E):
        t1 = np.nonzero(i1 == e)[0]
        t2 = np.nonzero(i2 == e)[0]
        idxs.append(np.concatenate([t1, t2]))
        wks.append(np.concatenate([tw1[t1], tw2[t2]]).astype(np.float32))
    max_n = max(len(ix) for ix in idxs)
    C = min(768, max(512, -(-max_n // 64) * 64))
    nc = _get_nc(C, USE_FP8)
    nct = -(-C // P)
    Cp = nct * P

    in_maps = []
    for e in range(E):
        n = min(len(idxs[e]), C)
        rows = idxs[e][:n]
        xg_e = np.zeros((Cp, D), dtype=np.float32)
        xg_e[:n] = xf[rows]
        # dg gate (scalar per token) on host; sc = dg * topk_weight
        dgv = 1.0 / (1.0 + np.exp(-(xg_e[:n] @ dg_w[e] + dg_b[e])))
        sc_e = np.zeros((Cp,), dtype=np.float32)
        sc_e[:n] = dgv * wks[e][:n]
        # fold w2 bias into the residual: y = (xg + sc*w2_b) + sc*ff
        xg2_e = xg_e + sc_e[:, None] * w2_b[e][None, :]
        if USE_FP8:
            s_x = _pow2_scale(np.abs(xg_e).max())
            s_w1 = _pow2_scale(np.abs(w1_w[e]).max())
            s_w3 = _pow2_scale(np.abs(w3_w[e]).max())
            xT_h = np.ascontiguousarray(
                (xg_e[:C] * s_x).T.reshape(KC, P, C).transpose(1, 0, 2)
            ).astype(NPFP8)
            desc_e = np.array([1.0 / (s_x * s_w1), 1.0 / (s_x * s_w3)],
                              dtype=np.float32)
        else:
            xT_h = np.ascontiguousarray(
                xg_e[:C].T.reshape(KC, P, C).transpose(1, 0, 2)).astype(NPBF16)
            s_w1 = s_w3 = 1.0
            desc_e = np.ones(2, dtype=np.float32)
        xg2_h = np.ascontiguousarray(
            xg2_e.reshape(nct, P, D).transpose(1, 0, 2))
        in_maps.append({
            "xT": xT_h,
            "xg": xg2_h,
            "sc": sc_e,
            "w1": _arr_w13(w1_w[e] * s_w1) if USE_FP8 else _arr_w13(w1_w[e]),
            "w3": _arr_w13(w3_w[e] * s_w3) if USE_FP8 else _arr_w13(w3_w[e]),
            "desc": desc_e,
            "w2": _arr_w2(w2_w[e]),
            "w1b": np.ascontiguousarray(w1_b[e]),
            "w3b": np.ascontiguousarray(w3_b[e]),
            "lng": np.ascontiguousarray(ln_g[e]),
            "lnb": np.ascontiguousarray(ln_b[e]),
        })

    trace = os.environ.get("MOE_TRACE", "0") == "1"
    if trace:
        _ensure_ntff_hook()
    res = run_bass_kernel_spmd(nc, in_maps, core_ids=list(range(E)),
                               trace=trace,
                               tmpdir=os.environ.get("MOE_TRACE_DIR") or None)
    global LAST_RESULTS
    LAST_RESULTS = res

    out = np.zeros((T, D), dtype=np.float32)
    for e in range(E):
        n = min(len(idxs[e]), C)
        out[idxs[e][:n]] += res.results[e]["yout"][:n]
        if len(idxs[e]) > C:
            # capacity overflow (pathological routing): exact host math
            ov = idxs[e][C:]
            ffv, dgv = _host_expert(xf[ov], e, w1_w, w1_b, w3_w, w3_b,
                                    w2_w, w2_b, dg_w, dg_b)
            y = xf[ov] + (dgv * wks[e][C:])[:, None] * ffv
            mu = y.mean(axis=-1, keepdims=True)
            var = np.square(y - mu).mean(axis=-1, keepdims=True)
            out[ov] += ((y - mu) / np.sqrt(var + LN_EPS)) * ln_g[e] + ln_b[e]

    expert_load = np.bincount(np.concatenate([i1, i2]),
                              minlength=E).astype(np.float32)
    return out.reshape(B, S, D), expert_load
